# revision 1
# baseline (speedup 1.0000x reference)
"""MultiHeadDecoder (moe_routing) Trainium2 kernel.

Strategy: expert-parallel. Each of the 8 cores owns one head's weights.
Host groups samples by head index, pads each group to a common capacity C,
and transposes X so the contraction dim lands on partitions. Each core runs
a dense 2-layer MLP (256->512 relu, 512->2048) for its head's samples.
Host scatters rows back to original order.

Layer 1 computes H^T (hid on partitions) so layer 2 can contract over hid
without an on-chip transpose:
  H^T[hc]  = W1[:, hc].T @ X^T      (lhsT=W1 chunk, rhs=X^T chunk)
  out[st]  = (H^T[:, st]).T @ W2    (lhsT=H^T chunk, rhs=W2 chunk)
"""

import numpy as np

import concourse.bass as bass
import concourse.mybir as mybir
from concourse import bacc
from concourse.tile import TileContext
from concourse.bass_utils import run_bass_kernel_spmd

IN_F, HID, OUT_F, N_HEADS, BATCH = 256, 512, 2048, 8, 4096
N_CORES = 8
P = 128

f32 = mybir.dt.float32

_NC_CACHE: dict = {}


def build_nc(C: int):
    """Build the per-core Bass program for capacity C (multiple of 128)."""
    KI = IN_F // P   # 2  input-feature chunks
    HC = HID // P    # 4  hidden chunks
    OC = OUT_F // 512  # 4 output-feature chunks of 512
    ST = C // P      # sample tiles

    nc = bacc.Bacc("TRN2", target_bir_lowering=False, debug=False,
                   num_devices=N_CORES)
    xt = nc.dram_tensor("xt", [IN_F, C], f32, kind="ExternalInput")
    w1 = nc.dram_tensor("w1", [IN_F, HID], f32, kind="ExternalInput")
    b1s = nc.dram_tensor("b1s", [P, HC], f32, kind="ExternalInput")
    w2 = nc.dram_tensor("w2", [HID, OUT_F], f32, kind="ExternalInput")
    b2r = nc.dram_tensor("b2r", [P, OUT_F], f32, kind="ExternalInput")
    out = nc.dram_tensor("out", [C, OUT_F], f32, kind="ExternalOutput")

    relu = mybir.ActivationFunctionType.Relu

    with TileContext(nc) as tc:
        with (
            tc.tile_pool(name="const", bufs=1) as const,
            tc.tile_pool(name="psumA", bufs=4, space="PSUM") as psumA,
            tc.tile_pool(name="psumB", bufs=4, space="PSUM") as psumB,
            tc.tile_pool(name="outp", bufs=3) as outp,
        ):
            xt_s = const.tile([P, KI, C], f32)
            nc.sync.dma_start(xt_s[:], xt.rearrange("(k p) c -> p k c", p=P))
            w1_s = const.tile([P, KI, HID], f32)
            nc.sync.dma_start(w1_s[:], w1.rearrange("(k p) h -> p k h", p=P))
            b1_s = const.tile([P, HC], f32)
            nc.sync.dma_start(b1_s[:], b1s[:])
            b2_s = const.tile([P, OUT_F], f32)
            nc.sync.dma_start(b2_s[:], b2r[:])
            w2_cs = []
            for oc in range(OC):
                w2_c = const.tile([P, HC, 512], f32, tag=f"w2_{oc}")
                nc.sync.dma_start(
                    w2_c[:],
                    w2.rearrange("(c p) o -> p c o", p=P)[:, :, oc * 512:(oc + 1) * 512],
                )
                w2_cs.append(w2_c)

            # Stage A: H^T [hid(part), sample(free)], relu(x @ W1 + b1)
            ht = const.tile([P, HC, C], f32)
            sgroups = [(s, min(512, C - s)) for s in range(0, C, 512)]
            for hc in range(HC):
                for (s0, sn) in sgroups:
                    ps = psumA.tile([P, 512], f32, tag="psA")
                    for k in range(KI):
                        nc.tensor.matmul(
                            ps[:, :sn],
                            lhsT=w1_s[:, k, hc * P:(hc + 1) * P],
                            rhs=xt_s[:, k, s0:s0 + sn],
                            start=(k == 0), stop=(k == KI - 1),
                        )
                    nc.scalar.activation(
                        ht[:, hc, s0:s0 + sn], ps[:, :sn], relu,
                        bias=b1_s[:, hc:hc + 1],
                    )

            # Stage B: out[st] = H[st] @ W2 + b2
            for st in range(ST):
                ot = outp.tile([P, OUT_F], f32, tag="ot")
                for oc in range(OC):
                    ps = psumB.tile([P, 512], f32, tag="psB")
                    for hc in range(HC):
                        nc.tensor.matmul(
                            ps[:],
                            lhsT=ht[:, hc, st * P:(st + 1) * P],
                            rhs=w2_cs[oc][:, hc, :],
                            start=(hc == 0), stop=(hc == HC - 1),
                        )
                    nc.vector.tensor_add(
                        out=ot[:, oc * 512:(oc + 1) * 512],
                        in0=ps[:],
                        in1=b2_s[:, oc * 512:(oc + 1) * 512],
                    )
                nc.sync.dma_start(out[st * P:(st + 1) * P, :], ot[:])

    nc.compile()
    return nc


def kernel(X, X_head_idx, W1, b1, W2, b2):
    X = np.ascontiguousarray(np.asarray(X, dtype=np.float32))
    idx = np.asarray(X_head_idx).astype(np.int64)
    W1 = np.asarray(W1, dtype=np.float32)
    b1 = np.asarray(b1, dtype=np.float32)
    W2 = np.asarray(W2, dtype=np.float32)
    b2 = np.asarray(b2, dtype=np.float32)

    batch = X.shape[0]
    counts = np.bincount(idx, minlength=N_HEADS)
    order = np.argsort(idx, kind="stable")
    positions = np.split(order, np.cumsum(counts)[:-1])

    C = max(512, int(-(-counts.max() // P)) * P)
    key = C
    if key not in _NC_CACHE:
        _NC_CACHE[key] = build_nc(C)
    nc = _NC_CACHE[key]

    in_maps = []
    for h in range(N_HEADS):
        pos = positions[h]
        xt = np.zeros((IN_F, C), dtype=np.float32)
        if len(pos):
            xt[:, :len(pos)] = X[pos].T
        in_maps.append({
            "xt": xt,
            "w1": np.ascontiguousarray(W1[h]),
            "b1s": np.ascontiguousarray(b1[h].reshape(HID // P, P).T),
            "w2": np.ascontiguousarray(W2[h]),
            "b2r": np.ascontiguousarray(np.broadcast_to(b2[h], (P, OUT_F))),
        })

    res = run_bass_kernel_spmd(nc, in_maps, list(range(N_CORES)))

    out = np.empty((batch, OUT_F), dtype=np.float32)
    for h in range(N_HEADS):
        pos = positions[h]
        if len(pos):
            out[pos] = res.results[h]["out"][:len(pos)]
    return out


# revision 3
# speedup vs baseline: 1.9540x; 1.9540x over previous
"""MultiHeadDecoder (moe_routing) Trainium2 kernel.

Strategy: expert-parallel. Each of the 8 cores owns one head's weights.
Host groups samples by head index, pads each group to a common capacity C,
and transposes X so the contraction dim lands on partitions. Each core runs
a dense 2-layer MLP (256->512 relu, 512->2048) for its head's samples.
Host scatters rows back to original order.

Layer 1 computes H^T (hid on partitions) so layer 2 can contract over hid
without an on-chip transpose:
  H^T[hc]  = W1[:, hc].T @ X^T      (lhsT=W1 chunk, rhs=X^T chunk)
  out[st]  = (H^T[:, st]).T @ W2    (lhsT=H^T chunk, rhs=W2 chunk)
"""

import numpy as np

import concourse.bass as bass
import concourse.mybir as mybir
from concourse import bacc
from concourse.tile import TileContext
from concourse.bass_utils import run_bass_kernel_spmd

IN_F, HID, OUT_F, N_HEADS, BATCH = 256, 512, 2048, 8, 4096
N_CORES = 8
P = 128

f32 = mybir.dt.float32
f32r = mybir.dt.float32r  # fp32 bits, PE runs at full (bf16) rate, tf32-ish mul

_NC_CACHE: dict = {}


def build_nc(C: int):
    """Build the per-core Bass program for capacity C (multiple of 128)."""
    KI = IN_F // P   # 2  input-feature chunks
    HC = HID // P    # 4  hidden chunks
    OC = OUT_F // 512  # 4 output-feature chunks of 512
    ST = C // P      # sample tiles

    nc = bacc.Bacc("TRN2", target_bir_lowering=False, debug=False,
                   num_devices=N_CORES)
    xt = nc.dram_tensor("xt", [IN_F, C], f32r, kind="ExternalInput")
    w1 = nc.dram_tensor("w1", [IN_F, HID], f32r, kind="ExternalInput")
    b1s = nc.dram_tensor("b1s", [P, HC], f32, kind="ExternalInput")
    w2 = nc.dram_tensor("w2", [HID, OUT_F], f32r, kind="ExternalInput")
    b2r = nc.dram_tensor("b2r", [P, OUT_F], f32, kind="ExternalInput")
    out = nc.dram_tensor("out", [C, OUT_F], f32, kind="ExternalOutput")

    relu = mybir.ActivationFunctionType.Relu

    with TileContext(nc) as tc:
        with (
            tc.tile_pool(name="const", bufs=1) as const,
            tc.tile_pool(name="psumA", bufs=4, space="PSUM") as psumA,
            tc.tile_pool(name="psumB", bufs=4, space="PSUM") as psumB,
            tc.tile_pool(name="outp", bufs=3) as outp,
        ):
            xt_s = const.tile([P, KI, C], f32r)
            nc.sync.dma_start(xt_s[:], xt.rearrange("(k p) c -> p k c", p=P))
            w1_s = const.tile([P, KI, HID], f32r)
            nc.sync.dma_start(w1_s[:], w1.rearrange("(k p) h -> p k h", p=P))
            b1_s = const.tile([P, HC], f32)
            nc.sync.dma_start(b1_s[:], b1s[:])
            b2_s = const.tile([P, OUT_F], f32)
            nc.sync.dma_start(b2_s[:], b2r[:])
            w2_cs = []
            for oc in range(OC):
                w2_c = const.tile([P, HC, 512], f32r, tag=f"w2_{oc}")
                nc.sync.dma_start(
                    w2_c[:],
                    w2.rearrange("(c p) o -> p c o", p=P)[:, :, oc * 512:(oc + 1) * 512],
                )
                w2_cs.append(w2_c)

            # Stage A: H^T [hid(part), sample(free)], relu(x @ W1 + b1)
            ht = const.tile([P, HC, C], f32r)
            sgroups = [(s, min(512, C - s)) for s in range(0, C, 512)]
            for hc in range(HC):
                for (s0, sn) in sgroups:
                    ps = psumA.tile([P, 512], f32, tag="psA")
                    for k in range(KI):
                        nc.tensor.matmul(
                            ps[:, :sn],
                            lhsT=w1_s[:, k, hc * P:(hc + 1) * P],
                            rhs=xt_s[:, k, s0:s0 + sn],
                            start=(k == 0), stop=(k == KI - 1),
                        )
                    nc.scalar.activation(
                        ht[:, hc, s0:s0 + sn], ps[:, :sn], relu,
                        bias=b1_s[:, hc:hc + 1],
                    )

            # Stage B: out[st] = H[st] @ W2 + b2
            for st in range(ST):
                ot = outp.tile([P, OUT_F], f32, tag="ot")
                for oc in range(OC):
                    ps = psumB.tile([P, 512], f32, tag="psB")
                    for hc in range(HC):
                        nc.tensor.matmul(
                            ps[:],
                            lhsT=ht[:, hc, st * P:(st + 1) * P],
                            rhs=w2_cs[oc][:, hc, :],
                            start=(hc == 0), stop=(hc == HC - 1),
                        )
                    nc.vector.tensor_add(
                        out=ot[:, oc * 512:(oc + 1) * 512],
                        in0=ps[:],
                        in1=b2_s[:, oc * 512:(oc + 1) * 512],
                    )
                nc.sync.dma_start(out[st * P:(st + 1) * P, :], ot[:])

    nc.compile()
    return nc


def kernel(X, X_head_idx, W1, b1, W2, b2):
    X = np.ascontiguousarray(np.asarray(X, dtype=np.float32))
    idx = np.asarray(X_head_idx).astype(np.int64)
    W1 = np.asarray(W1, dtype=np.float32)
    b1 = np.asarray(b1, dtype=np.float32)
    W2 = np.asarray(W2, dtype=np.float32)
    b2 = np.asarray(b2, dtype=np.float32)

    batch = X.shape[0]
    counts = np.bincount(idx, minlength=N_HEADS)
    order = np.argsort(idx, kind="stable")
    positions = np.split(order, np.cumsum(counts)[:-1])

    C = max(512, int(-(-counts.max() // P)) * P)
    key = C
    if key not in _NC_CACHE:
        _NC_CACHE[key] = build_nc(C)
    nc = _NC_CACHE[key]

    in_maps = []
    for h in range(N_HEADS):
        pos = positions[h]
        xt = np.zeros((IN_F, C), dtype=np.float32)
        if len(pos):
            xt[:, :len(pos)] = X[pos].T
        in_maps.append({
            "xt": xt,
            "w1": np.ascontiguousarray(W1[h]),
            "b1s": np.ascontiguousarray(b1[h].reshape(HID // P, P).T),
            "w2": np.ascontiguousarray(W2[h]),
            "b2r": np.ascontiguousarray(np.broadcast_to(b2[h], (P, OUT_F))),
        })

    res = run_bass_kernel_spmd(nc, in_maps, list(range(N_CORES)))

    out = np.empty((batch, OUT_F), dtype=np.float32)
    for h in range(N_HEADS):
        pos = positions[h]
        if len(pos):
            out[pos] = res.results[h]["out"][:len(pos)]
    return out


# revision 4
# speedup vs baseline: 2.1382x; 1.0942x over previous
"""MultiHeadDecoder (moe_routing) Trainium2 kernel.

Strategy: expert-parallel. Each of the 8 cores owns one head's weights.
Host groups samples by head index, pads each group to a common capacity C,
and transposes X so the contraction dim lands on partitions. Each core runs
a dense 2-layer MLP (256->512 relu, 512->2048) for its head's samples.
Host scatters rows back to original order.

Layer 1 computes H^T (hid on partitions) so layer 2 can contract over hid
without an on-chip transpose:
  H^T[hc]  = W1[:, hc].T @ X^T      (lhsT=W1 chunk, rhs=X^T chunk)
  out[st]  = (H^T[:, st]).T @ W2    (lhsT=H^T chunk, rhs=W2 chunk)

Matmuls run in float32r (fp32 bits, full PE rate, tf32-ish multiply).
Stage B is ordered oc-outer so only the first W2 chunk's DMA gates its
start; outputs stream out per (oc, st) slice.
"""

import numpy as np

import concourse.bass as bass
import concourse.mybir as mybir
from concourse import bacc
from concourse.tile import TileContext
from concourse.bass_utils import run_bass_kernel_spmd

IN_F, HID, OUT_F, N_HEADS, BATCH = 256, 512, 2048, 8, 4096
N_CORES = 8
P = 128

f32 = mybir.dt.float32
f32r = mybir.dt.float32r  # fp32 bits, PE runs at full (bf16) rate, tf32-ish mul

_NC_CACHE: dict = {}


def build_nc(C: int):
    """Build the per-core Bass program for capacity C (multiple of 128)."""
    KI = IN_F // P   # 2  input-feature chunks
    HC = HID // P    # 4  hidden chunks
    OC = OUT_F // 512  # 4 output-feature chunks of 512
    ST = C // P      # sample tiles

    nc = bacc.Bacc("TRN2", target_bir_lowering=False, debug=False,
                   num_devices=N_CORES)
    xt = nc.dram_tensor("xt", [IN_F, C], f32r, kind="ExternalInput")
    w1 = nc.dram_tensor("w1", [IN_F, HID], f32r, kind="ExternalInput")
    b1s = nc.dram_tensor("b1s", [P, HC], f32, kind="ExternalInput")
    w2 = nc.dram_tensor("w2", [HID, OUT_F], f32r, kind="ExternalInput")
    b2 = nc.dram_tensor("b2", [1, OUT_F], f32, kind="ExternalInput")
    out = nc.dram_tensor("out", [C, OUT_F], f32, kind="ExternalOutput")

    relu = mybir.ActivationFunctionType.Relu

    with TileContext(nc) as tc:
        with (
            tc.tile_pool(name="const", bufs=1) as const,
            tc.tile_pool(name="psumA", bufs=2, space="PSUM") as psumA,
            tc.tile_pool(name="psumB", bufs=4, space="PSUM") as psumB,
            tc.tile_pool(name="outp", bufs=6) as outp,
        ):
            # Stage-A inputs first so the PE starts ASAP; W2 streams behind.
            xt_s = const.tile([P, KI, C], f32r)
            nc.sync.dma_start(xt_s[:], xt.rearrange("(k p) c -> p k c", p=P))
            w1_s = const.tile([P, KI, HID], f32r)
            nc.sync.dma_start(w1_s[:], w1.rearrange("(k p) h -> p k h", p=P))
            b1_s = const.tile([P, HC], f32)
            nc.sync.dma_start(b1_s[:], b1s[:])
            b2_row = const.tile([1, OUT_F], f32)
            nc.sync.dma_start(b2_row[:], b2[:])
            b2_s = const.tile([P, OUT_F], f32)
            nc.gpsimd.partition_broadcast(b2_s[:], b2_row[:])
            w2_cs = []
            for oc in range(OC):
                w2_c = const.tile([P, HC, 512], f32r, tag=f"w2_{oc}")
                nc.sync.dma_start(
                    w2_c[:],
                    w2.rearrange("(c p) o -> p c o", p=P)[:, :, oc * 512:(oc + 1) * 512],
                )
                w2_cs.append(w2_c)

            # Stage A: H^T [hid(part), sample(free)], relu(x @ W1 + b1)
            ht = const.tile([P, HC, C], f32r)
            sgroups = [(s, min(512, C - s)) for s in range(0, C, 512)]
            for hc in range(HC):
                for (s0, sn) in sgroups:
                    ps = psumA.tile([P, 512], f32, tag="psA")
                    for k in range(KI):
                        nc.tensor.matmul(
                            ps[:, :sn],
                            lhsT=w1_s[:, k, hc * P:(hc + 1) * P],
                            rhs=xt_s[:, k, s0:s0 + sn],
                            start=(k == 0), stop=(k == KI - 1),
                        )
                    nc.scalar.activation(
                        ht[:, hc, s0:s0 + sn], ps[:, :sn], relu,
                        bias=b1_s[:, hc:hc + 1],
                    )

            # Stage B: out[st, oc] = H[st] @ W2[:, oc] + b2[oc]
            for oc in range(OC):
                for st in range(ST):
                    ps = psumB.tile([P, 512], f32, tag="psB")
                    for hc in range(HC):
                        nc.tensor.matmul(
                            ps[:],
                            lhsT=ht[:, hc, st * P:(st + 1) * P],
                            rhs=w2_cs[oc][:, hc, :],
                            start=(hc == 0), stop=(hc == HC - 1),
                        )
                    ot = outp.tile([P, 512], f32, tag="ot")
                    nc.vector.tensor_add(
                        out=ot[:],
                        in0=ps[:],
                        in1=b2_s[:, oc * 512:(oc + 1) * 512],
                    )
                    nc.sync.dma_start(
                        out[st * P:(st + 1) * P, oc * 512:(oc + 1) * 512], ot[:]
                    )

    nc.compile()
    return nc


def kernel(X, X_head_idx, W1, b1, W2, b2):
    X = np.ascontiguousarray(np.asarray(X, dtype=np.float32))
    idx = np.asarray(X_head_idx).astype(np.int64)
    W1 = np.asarray(W1, dtype=np.float32)
    b1 = np.asarray(b1, dtype=np.float32)
    W2 = np.asarray(W2, dtype=np.float32)
    b2 = np.asarray(b2, dtype=np.float32)

    batch = X.shape[0]
    counts = np.bincount(idx, minlength=N_HEADS)
    order = np.argsort(idx, kind="stable")
    positions = np.split(order, np.cumsum(counts)[:-1])

    C = max(512, int(-(-counts.max() // P)) * P)
    key = C
    if key not in _NC_CACHE:
        _NC_CACHE[key] = build_nc(C)
    nc = _NC_CACHE[key]

    in_maps = []
    for h in range(N_HEADS):
        pos = positions[h]
        xt = np.zeros((IN_F, C), dtype=np.float32)
        if len(pos):
            xt[:, :len(pos)] = X[pos].T
        in_maps.append({
            "xt": xt,
            "w1": np.ascontiguousarray(W1[h]),
            "b1s": np.ascontiguousarray(b1[h].reshape(HID // P, P).T),
            "w2": np.ascontiguousarray(W2[h]),
            "b2": np.ascontiguousarray(b2[h][None, :]),
        })

    res = run_bass_kernel_spmd(nc, in_maps, list(range(N_CORES)))

    out = np.empty((batch, OUT_F), dtype=np.float32)
    for h in range(N_HEADS):
        pos = positions[h]
        if len(pos):
            out[pos] = res.results[h]["out"][:len(pos)]
    return out
